# revision 5
# baseline (speedup 1.0000x reference)
"""Trainium2 Bass kernel for ClippingAttentionEngine.

Sharding: core c -> (batch b = c//2, head-group hg = c%2, 8 heads each).
Each core computes Q/K/V projections for its 8 heads, attention, and the
partial output projection over its head slice; host sums the two per-batch
partials (tensor-parallel over heads, per the sharding hint).

The per-batch sparse/dense branch is folded into a single dense-shaped
program: a host-built additive bias matrix B encodes either the dense
prior bias (0 / -lambda) or, for sparse batches, log(multiplicity) of each
key under prior_indices/prior_index_mask (-1e4 where never indexed), which
makes dense softmax(QK^T*scale + B) @ V exactly reproduce the gathered
sparse softmax (duplicates included).
"""

import sys

sys.path.insert(0, "/opt/trn_rl_repo")

import numpy as np

import concourse.bass as bass
import concourse.tile as tile
from concourse import bacc, mybir
from concourse.alu_op_type import AluOpType
from concourse.bass_utils import run_bass_kernel_spmd

B, S, D, H = 4, 1024, 1024, 16
DH = D // H          # 64
HPC = 8              # heads per core
MPAIR = HPC // 2     # head pairs per core
N_CORES = 8
KT = S // 128        # 8 k tiles
QT = S // 512        # 2 q tiles
DCH = D // 128       # 8 contraction chunks
LAMBDA_MAX, ALPHA, SPARSE_THRESHOLD = 10.0, 5.0, 1.0

F32 = mybir.dt.float32
F32R = mybir.dt.float32r
EXP = mybir.ActivationFunctionType.Exp
IDENT = mybir.ActivationFunctionType.Identity


def _r(ap):
    return ap.bitcast(F32R)


def build_program():
    nc = bacc.Bacc("TRN2", target_bir_lowering=False, debug=False,
                   num_devices=N_CORES)

    d_xt = nc.dram_tensor("xt", [D, S], F32R, kind="ExternalInput").ap()
    d_wqt = nc.dram_tensor("wqt", [D, 512], F32R, kind="ExternalInput").ap()
    d_wkt = nc.dram_tensor("wkt", [D, 512], F32R, kind="ExternalInput").ap()
    d_wvt = nc.dram_tensor("wvt", [D, 512], F32R, kind="ExternalInput").ap()
    d_wot = nc.dram_tensor("wot", [512, D], F32R, kind="ExternalInput").ap()
    d_bt = nc.dram_tensor("bt", [S, S], F32R, kind="ExternalInput").ap()
    d_bq = nc.dram_tensor("bq", [128, 4], F32, kind="ExternalInput").ap()
    d_bk = nc.dram_tensor("bk", [128, 4], F32, kind="ExternalInput").ap()
    d_bv = nc.dram_tensor("bv", [1, 512], F32R, kind="ExternalInput").ap()
    d_bo = nc.dram_tensor("bo", [1, D], F32R, kind="ExternalInput").ap()
    d_id = nc.dram_tensor("ident", [128, 128], F32R, kind="ExternalInput").ap()
    d_ones = nc.dram_tensor("ones", [128, 512], F32R, kind="ExternalInput").ap()
    d_out = nc.dram_tensor("out", [S, D], F32, kind="ExternalOutput").ap()

    with tile.TileContext(nc) as tc:
        with (
            tc.tile_pool(name="const", bufs=1) as constp,
            tc.tile_pool(name="main", bufs=1) as mainp,
            tc.tile_pool(name="outp", bufs=2) as outp,
            tc.tile_pool(name="psA", bufs=4, space="PSUM") as psA,
            tc.tile_pool(name="psB", bufs=2, space="PSUM") as psB,
        ):
            ident = constp.tile([128, 128], F32R, tag="ident")
            nc.sync.dma_start(ident[:], d_id[:])
            ones = constp.tile([1, 512], F32R, tag="ones")
            nc.sync.dma_start(ones[:], d_ones[0:1, :])
            onecol = constp.tile([128, 8], F32R, tag="onecol")
            nc.sync.dma_start(onecol[:], d_ones[:, 0:8])
            bq_sb = constp.tile([128, 4], F32, tag="bq")
            nc.sync.dma_start(bq_sb[:], d_bq[:])
            bk_sb = constp.tile([128, 4], F32, tag="bk")
            nc.sync.dma_start(bk_sb[:], d_bk[:])
            bv_sb = constp.tile([1, 512], F32R, tag="bv")
            nc.sync.dma_start(bv_sb[:], d_bv[:])
            bo_sb = constp.tile([1, D], F32R, tag="bo")
            nc.sync.dma_start(bo_sb[:], d_bo[:])

            # Persistent arrays.
            qt_sb = [mainp.tile([128, S], F32R, tag=f"qt{m}", name=f"qt{m}") for m in range(4)]
            kt_sb = [mainp.tile([128, S], F32R, tag=f"kt{m}", name=f"kt{m}") for m in range(4)]
            vp_sb = [mainp.tile([128, HPC * (DH + 1)], F32R, tag=f"vp{sb}", name=f"vp{sb}")
                     for sb in range(8)]
            at_sb = [mainp.tile([128, S], F32R, tag=f"at{m}", name=f"at{m}") for m in range(4)]
            wot_sb = [mainp.tile([128, D], F32R, tag=f"wot{mc}", name=f"wot{mc}") for mc in range(4)]
            for mc in range(4):
                nc.sync.dma_start(wot_sb[mc][:], d_wot[mc * 128:(mc + 1) * 128, :])

            # ---- Stage A: projections (scoped pool: xT + W slices) ----
            with tc.tile_pool(name="stageA", bufs=1) as pA:
                xt_sb = [pA.tile([128, S], F32R, tag=f"xt{c}", name=f"xt{c}") for c in range(DCH)]
                for c in range(DCH):
                    nc.sync.dma_start(xt_sb[c][:], d_xt[c * 128:(c + 1) * 128, :])
                w_sb = {}
                for nm, dap in (("q", d_wqt), ("k", d_wkt), ("v", d_wvt)):
                    w_sb[nm] = [pA.tile([128, 512], F32R, tag=f"w{nm}{c}", name=f"w{nm}{c}")
                                for c in range(DCH)]
                    for c in range(DCH):
                        nc.sync.dma_start(w_sb[nm][c][:],
                                          dap[c * 128:(c + 1) * 128, :])

                # Q^T and K^T: psum[d'128, s512] = sum_c W^T[c][:,d'].T @ xT[c][:,s]
                for nm, dst, bias in (("q", qt_sb, bq_sb), ("k", kt_sb, bk_sb)):
                    for m in range(4):
                        for st in range(2):
                            ps = psA.tile([128, 512], F32, tag="ps")
                            for c in range(DCH):
                                nc.tensor.matmul(
                                    ps[:],
                                    (w_sb[nm][c][:, m * 128:(m + 1) * 128]),
                                    (xt_sb[c][:, st * 512:(st + 1) * 512]),
                                    start=(c == 0), stop=(c == DCH - 1))
                            # copy PSUM -> SBUF with per-partition bias add
                            nc.scalar.activation(
                                dst[m][:, st * 512:(st + 1) * 512], ps[:],
                                IDENT, bias=bias[:, m:m + 1])

                # V natural: psum[s128, dh512] = sum_c xT[c][:,sblk].T @ WvT[c]
                for sb in range(8):
                    ps = psA.tile([128, 512], F32, tag="ps")
                    for c in range(DCH):
                        nc.tensor.matmul(
                            ps[:],
                            (xt_sb[c][:, sb * 128:(sb + 1) * 128]),
                            (w_sb["v"][c][:]),
                            start=(c == 0), stop=False)
                    # + bv broadcast over rows (rank-1 via ones)
                    nc.tensor.matmul(ps[:], (ones[0:1, 0:128]), (bv_sb[:]),
                                     start=False, stop=True)
                    vp3 = vp_sb[sb].rearrange("p (h d) -> p h d", d=DH + 1)
                    nc.vector.tensor_copy(
                        vp3[:, :, 0:DH],
                        ps[:].rearrange("p (h d) -> p h d", d=DH))
                    nc.vector.tensor_copy(vp3[:, :, DH:DH + 1],
                                          onecol[:].rearrange("p (h o) -> p h o", o=1))

            # ---- Stage B: attention ----
            with (
                tc.tile_pool(name="stageB", bufs=1) as pB,
                tc.tile_pool(name="ptp", bufs=20) as ptp,
                tc.tile_pool(name="smallp", bufs=4) as smallp,
            ):
                bt_sb = [pB.tile([128, S], F32R, tag=f"bt{k}", name=f"bt{k}") for k in range(KT)]
                for k in range(KT):
                    nc.sync.dma_start(bt_sb[k][:], d_bt[k * 128:(k + 1) * 128, :])

                for m in range(MPAIR):
                    for q in range(QT):
                        pts = {}
                        for k in range(KT):
                            for hh in range(2):
                                ps = psA.tile([128, 512], F32, tag="ps")
                                # additive bias via identity matmul
                                nc.tensor.matmul(
                                    ps[:], (ident[:]),
                                    (bt_sb[k][:, q * 512:(q + 1) * 512]),
                                    start=True, stop=False)
                                # scores^T += K_h^T.T @ Q_h^T  (K=64 strip)
                                nc.tensor.matmul(
                                    ps[:],
                                    (kt_sb[m][hh * 64:(hh + 1) * 64,
                                                k * 128:(k + 1) * 128]),
                                    (qt_sb[m][hh * 64:(hh + 1) * 64,
                                                q * 512:(q + 1) * 512]),
                                    start=False, stop=True,
                                    tile_position=(hh * 64, 0))
                                pt = ptp.tile([128, 512], F32R, tag="pt")
                                nc.scalar.activation(pt[:], ps[:], EXP)
                                pts[(hh, k)] = pt
                        for hh in range(2):
                            h = m * 2 + hh
                            po = psB.tile([DH + 1, 512], F32, tag="att")
                            for k in range(KT):
                                nc.tensor.matmul(
                                    po[:],
                                    (vp_sb[k][:, h * (DH + 1):(h + 1) * (DH + 1)]),
                                    (pts[(hh, k)][:]),
                                    start=(k == 0), stop=(k == KT - 1))
                            rec = smallp.tile([1, 512], F32, tag="rec")
                            nc.vector.reciprocal(rec[:], po[DH:DH + 1, :])
                            bc = smallp.tile([64, 512], F32, tag="bc")
                            nc.gpsimd.partition_broadcast(bc[:], rec[:])
                            nc.vector.tensor_tensor(
                                at_sb[m][hh * 64:(hh + 1) * 64,
                                         q * 512:(q + 1) * 512],
                                po[0:DH, :], bc[:], AluOpType.mult)

            # ---- Stage C: partial output projection ----
            for sb in range(8):
                ot = outp.tile([128, D], F32, tag="ot")
                for q in range(2):
                    ps = psA.tile([128, 512], F32, tag="ps")
                    for mc in range(4):
                        nc.tensor.matmul(
                            ps[:],
                            (at_sb[mc][:, sb * 128:(sb + 1) * 128]),
                            (wot_sb[mc][:, q * 512:(q + 1) * 512]),
                            start=(mc == 0), stop=False)
                    nc.tensor.matmul(ps[:], (ones[0:1, 0:128]),
                                     (bo_sb[0:1, q * 512:(q + 1) * 512]),
                                     start=False, stop=True)
                    nc.scalar.copy(ot[:, q * 512:(q + 1) * 512], ps[:])
                nc.sync.dma_start(d_out[sb * 128:(sb + 1) * 128, :], ot[:])

    nc.compile()
    return nc


_prog = None


def _get_prog():
    global _prog
    if _prog is None:
        _prog = build_program()
    return _prog


def _host_prep(x, prior_mask, prior_indices, prior_index_mask, u_prev,
               Wq, bq, Wk, bk, Wv, bv, Wo, bo):
    f32 = np.float32
    x = np.asarray(x, f32)
    pm = np.asarray(prior_mask, bool)
    idx = np.asarray(prior_indices)
    pim = np.asarray(prior_index_mask, bool)
    u = np.asarray(u_prev, f32).reshape(B)
    Wq, Wk, Wv, Wo = (np.asarray(w, f32) for w in (Wq, Wk, Wv, Wo))
    bq, bk, bv, bo = (np.asarray(v, f32) for v in (bq, bk, bv, bo))

    scale = f32(1.0 / np.sqrt(DH))
    lam = (LAMBDA_MAX * np.exp(-ALPHA * u.astype(np.float64))).astype(f32)
    use_sparse = lam >= SPARSE_THRESHOLD

    # Sparse multiplicity bias (shared across batches): log(count) or -1e4.
    bts_sparse = None
    if use_sparse.any():
        cnt = np.zeros((S, S + 1), np.int32)
        np.add.at(cnt, (np.arange(S)[:, None],
                        np.where(pim, idx, S).astype(np.int64)), 1)
        cnt = cnt[:, :S]
        bsp = np.where(cnt > 0, np.log(np.maximum(cnt, 1)).astype(f32),
                       f32(-10000.0))
        bts_sparse = np.ascontiguousarray(bsp.T)

    bts = []
    for b in range(B):
        if use_sparse[b]:
            bts.append(bts_sparse)
        else:
            bd = np.where(pm, f32(0.0), f32(-lam[b]))
            bts.append(np.ascontiguousarray(bd.T))

    in_maps = []
    for c in range(N_CORES):
        b = c // 2
        hg = c % 2
        hsl = slice(hg * 512, (hg + 1) * 512)
        in_maps.append({
            "xt": np.ascontiguousarray(x[b].T),
            "wqt": np.ascontiguousarray((Wq[hsl] * scale).T),
            "wkt": np.ascontiguousarray(Wk[hsl].T),
            "wvt": np.ascontiguousarray(Wv[hsl].T),
            "wot": np.ascontiguousarray(Wo[:, hsl].T),
            "bt": bts[b],
            "bq": np.ascontiguousarray((bq[hsl] * scale).reshape(4, 128).T),
            "bk": np.ascontiguousarray(bk[hsl].reshape(4, 128).T),
            "bv": np.ascontiguousarray(bv[hsl].reshape(1, 512)),
            "bo": np.ascontiguousarray((bo * f32(0.5)).reshape(1, D)),
            "ident": np.eye(128, dtype=f32),
            "ones": np.ones((128, 512), dtype=f32),
        })
    return in_maps


def kernel(**inputs):
    in_maps = _host_prep(**inputs)
    nc = _get_prog()
    res = run_bass_kernel_spmd(nc, in_maps, core_ids=list(range(N_CORES)))
    out = np.empty((B, S, D), np.float32)
    for b in range(B):
        out[b] = res.results[2 * b]["out"] + res.results[2 * b + 1]["out"]
    return out


# revision 10
# speedup vs baseline: 1.2281x; 1.2281x over previous
"""Trainium2 Bass kernel for ClippingAttentionEngine.

Sharding: core c -> (batch b = c//2, head-group hg = c%2, 8 heads each).
Each core computes Q/K/V projections for its 8 heads, attention, and the
partial output projection over its head slice; host sums the two per-batch
partials (tensor-parallel over heads, per the sharding hint).

The per-batch sparse/dense branch is folded into a single dense-shaped
program: a host-built additive bias matrix B encodes either the dense
prior bias (0 / -lambda) or, for sparse batches, log(multiplicity) of each
key under prior_indices/prior_index_mask (-1e4 where never indexed), which
makes dense softmax(QK^T*scale + B) @ V exactly reproduce the gathered
sparse softmax (duplicates included).

Device pipeline per core (all matmuls float32r):
  A) xT/W streamed in; Q^T,K^T ([d',s] layout) and V (natural, with a
     ones-column per head for the softmax denominator) projected.
  B) per (head-pair, q-half) group: scores^T = B^T (via identity matmul)
     + K^T.T @ Q^T accumulated in PSUM, exp on ACT into P^T tiles; the
     attn@V' accumulation + normalization of the PREVIOUS group is
     software-pipelined against the current group's scores.
  C) partial out-projection from the normalized attn^T tiles.
"""

import sys

sys.path.insert(0, "/opt/trn_rl_repo")

import numpy as np

import concourse.bass as bass
import concourse.tile as tile
from concourse import bacc, mybir
from concourse.alu_op_type import AluOpType
from concourse.bass_utils import run_bass_kernel_spmd

B, S, D, H = 4, 1024, 1024, 16
DH = D // H          # 64
HPC = 8              # heads per core
N_CORES = 8
KT = S // 128        # 8 k tiles
DCH = D // 128       # 8 contraction chunks
LAMBDA_MAX, ALPHA, SPARSE_THRESHOLD = 10.0, 5.0, 1.0

F32 = mybir.dt.float32
F32R = mybir.dt.float32r
EXP = mybir.ActivationFunctionType.Exp
IDENT = mybir.ActivationFunctionType.Identity


def build_program():
    nc = bacc.Bacc("TRN2", target_bir_lowering=False, debug=False,
                   num_devices=N_CORES)

    d_xt = nc.dram_tensor("xt", [D, S], F32R, kind="ExternalInput").ap()
    d_wqt = nc.dram_tensor("wqt", [D, 512], F32R, kind="ExternalInput").ap()
    d_wkt = nc.dram_tensor("wkt", [D, 512], F32R, kind="ExternalInput").ap()
    d_wvt = nc.dram_tensor("wvt", [D, 512], F32R, kind="ExternalInput").ap()
    d_wot = nc.dram_tensor("wot", [512, D], F32R, kind="ExternalInput").ap()
    d_bt = nc.dram_tensor("bt", [S, S], F32R, kind="ExternalInput").ap()
    d_bq = nc.dram_tensor("bq", [128, 4], F32, kind="ExternalInput").ap()
    d_bk = nc.dram_tensor("bk", [128, 4], F32, kind="ExternalInput").ap()
    d_bv = nc.dram_tensor("bv", [1, 512], F32R, kind="ExternalInput").ap()
    d_bo = nc.dram_tensor("bo", [1, D], F32R, kind="ExternalInput").ap()
    d_id = nc.dram_tensor("ident", [128, 128], F32R, kind="ExternalInput").ap()
    d_ones = nc.dram_tensor("ones", [128, 512], F32R, kind="ExternalInput").ap()
    d_out = nc.dram_tensor("out", [S, D], F32, kind="ExternalOutput").ap()

    with tile.TileContext(nc) as tc:
        with (
            tc.tile_pool(name="const", bufs=1) as constp,
            tc.tile_pool(name="main", bufs=1) as mainp,
        ):
            ident = constp.tile([128, 128], F32R, tag="ident")
            nc.sync.dma_start(ident[:], d_id[:])
            ones = constp.tile([1, 512], F32R, tag="ones")
            nc.sync.dma_start(ones[:], d_ones[0:1, :])
            onecol = constp.tile([128, 8], F32R, tag="onecol")
            nc.sync.dma_start(onecol[:], d_ones[:, 0:8])
            bq_sb = constp.tile([128, 4], F32, tag="bq")
            nc.sync.dma_start(bq_sb[:], d_bq[:])
            bk_sb = constp.tile([128, 4], F32, tag="bk")
            nc.sync.dma_start(bk_sb[:], d_bk[:])
            bv_sb = constp.tile([1, 512], F32R, tag="bv")
            nc.sync.dma_start(bv_sb[:], d_bv[:])
            bo_sb = constp.tile([1, D], F32R, tag="bo")
            nc.sync.dma_start(bo_sb[:], d_bo[:])

            # Persistent arrays.
            qt_sb = [mainp.tile([128, S], F32R, tag=f"qt{m}", name=f"qt{m}")
                     for m in range(4)]
            kt_sb = [mainp.tile([128, S], F32R, tag=f"kt{m}", name=f"kt{m}")
                     for m in range(4)]
            vp_sb = [mainp.tile([128, HPC * (DH + 1)], F32R, tag=f"vp{sb}",
                                name=f"vp{sb}") for sb in range(8)]
            at_sb = [mainp.tile([128, S], F32R, tag=f"at{m}", name=f"at{m}")
                     for m in range(4)]
            wot_sb = [mainp.tile([128, D], F32R, tag=f"wot{mc}", name=f"wot{mc}")
                      for mc in range(4)]
            bt_sb = [mainp.tile([128, S], F32R, tag=f"bt{k}", name=f"bt{k}")
                     for k in range(KT)]
            for mc in range(4):
                nc.sync.dma_start(wot_sb[mc][:], d_wot[mc * 128:(mc + 1) * 128, :])
            for k in range(KT):
                nc.sync.dma_start(bt_sb[k][:], d_bt[k * 128:(k + 1) * 128, :])

            # ---- Stage A: projections (scoped: xT + W slices + wide psum) ----
            with (
                tc.tile_pool(name="stageA", bufs=1) as pA,
                tc.tile_pool(name="ppp", bufs=2, space="PSUM") as ppp,
            ):
                xt_sb = [pA.tile([128, S], F32R, tag=f"xt{c}", name=f"xt{c}")
                         for c in range(DCH)]
                for c in range(DCH):
                    nc.sync.dma_start(xt_sb[c][:], d_xt[c * 128:(c + 1) * 128, :])
                w_sb = {}
                for nm, dap in (("q", d_wqt), ("k", d_wkt), ("v", d_wvt)):
                    w_sb[nm] = [pA.tile([128, 512], F32R, tag=f"w{nm}{c}",
                                        name=f"w{nm}{c}") for c in range(DCH)]
                    for c in range(DCH):
                        nc.sync.dma_start(w_sb[nm][c][:],
                                          dap[c * 128:(c + 1) * 128, :])

                # Q^T / K^T: psum[d'128, s1024] = sum_c W^T[c][:,d'].T @ xT[c]
                for nm, dst, bias in (("q", qt_sb, bq_sb), ("k", kt_sb, bk_sb)):
                    for m in range(4):
                        pp = ppp.tile([128, 1024], F32, tag="pp")
                        for st in range(2):
                            for c in range(DCH):
                                nc.tensor.matmul(
                                    pp[:, st * 512:(st + 1) * 512],
                                    w_sb[nm][c][:, m * 128:(m + 1) * 128],
                                    xt_sb[c][:, st * 512:(st + 1) * 512],
                                    start=(c == 0), stop=(c == DCH - 1))
                        nc.scalar.activation(dst[m][:], pp[:],
                                             IDENT, bias=bias[:, m:m + 1])

                # V natural: psum[s128, dh512] = sum_c xT[c][:,sblk].T @ WvT[c]
                for sb in range(8):
                    pp = ppp.tile([128, 1024], F32, tag="pp")
                    ps = pp[:, 0:512]
                    for c in range(DCH):
                        nc.tensor.matmul(
                            ps,
                            xt_sb[c][:, sb * 128:(sb + 1) * 128],
                            w_sb["v"][c][:],
                            start=(c == 0), stop=False)
                    nc.tensor.matmul(ps, ones[0:1, 0:128], bv_sb[:],
                                     start=False, stop=True)
                    vp3 = vp_sb[sb].rearrange("p (h d) -> p h d", d=DH + 1)
                    nc.vector.tensor_copy(
                        vp3[:, :, 0:DH],
                        ps.rearrange("p (h d) -> p h d", d=DH))
                    nc.vector.tensor_copy(
                        vp3[:, :, DH:DH + 1],
                        onecol[:].rearrange("p (h o) -> p h o", o=1))

            # ---- Stage B: attention, software-pipelined by (pair, q-half) --
            with (
                tc.tile_pool(name="ptp", bufs=28) as ptp,
                tc.tile_pool(name="smallp", bufs=2) as smallp,
                tc.tile_pool(name="outp", bufs=2) as outp,
                tc.tile_pool(name="psS", bufs=4, space="PSUM") as psS,
                tc.tile_pool(name="psO", bufs=4, space="PSUM") as psO,
            ):
                def emit_scores(m, q):
                    pts = {}
                    for k in range(KT):
                        for hh in range(2):
                            ps = psS.tile([128, 512], F32, tag="ps")
                            nc.tensor.matmul(
                                ps[:], ident[:],
                                bt_sb[k][:, q * 512:(q + 1) * 512],
                                start=True, stop=False)
                            nc.tensor.matmul(
                                ps[:],
                                kt_sb[m][hh * 64:(hh + 1) * 64,
                                         k * 128:(k + 1) * 128],
                                qt_sb[m][hh * 64:(hh + 1) * 64,
                                         q * 512:(q + 1) * 512],
                                start=False, stop=True,
                                tile_position=(hh * 64, 0))
                            pt = ptp.tile([128, 512], F32R, tag="pt")
                            nc.scalar.activation(pt[:], ps[:], EXP)
                            pts[(hh, k)] = pt
                    return pts

                def emit_attnv(m, q, pts):
                    for hh in range(2):
                        h = m * 2 + hh
                        po = psO.tile([DH + 1, 512], F32, tag="att")
                        for k in range(KT):
                            nc.tensor.matmul(
                                po[:],
                                vp_sb[k][:, h * (DH + 1):(h + 1) * (DH + 1)],
                                pts[(hh, k)][:],
                                start=(k == 0), stop=(k == KT - 1))
                        zrow = smallp.tile([1, 512], F32, tag="zrow")
                        nc.vector.tensor_copy(zrow[:], po[DH:DH + 1, :])
                        rec = smallp.tile([1, 512], F32, tag="rec")
                        scr = smallp.tile([1, 512], F32, tag="scr")
                        nc.vector.reciprocal_approx_accurate(
                            rec[:], zrow[:], scr[:])
                        bc = smallp.tile([64, 512], F32, tag="bc")
                        nc.gpsimd.partition_broadcast(bc[:], rec[:])
                        nc.vector.tensor_tensor(
                            at_sb[m][hh * 64:(hh + 1) * 64,
                                     q * 512:(q + 1) * 512],
                            po[0:DH, :], bc[:], AluOpType.mult)

                groups = [(m, q) for m in range(4) for q in range(2)]
                prev = None
                for g in groups:
                    pts = emit_scores(*g)
                    if prev is not None:
                        emit_attnv(prev[0][0], prev[0][1], prev[1])
                    prev = (g, pts)
                emit_attnv(prev[0][0], prev[0][1], prev[1])

                # ---- Stage C: partial output projection ----
                for sb in range(8):
                    ot = outp.tile([128, D], F32, tag="ot")
                    for q in range(2):
                        ps = psS.tile([128, 512], F32, tag="ps")
                        for mc in range(4):
                            nc.tensor.matmul(
                                ps[:],
                                at_sb[mc][:, sb * 128:(sb + 1) * 128],
                                wot_sb[mc][:, q * 512:(q + 1) * 512],
                                start=(mc == 0), stop=False)
                        nc.tensor.matmul(ps[:], ones[0:1, 0:128],
                                         bo_sb[0:1, q * 512:(q + 1) * 512],
                                         start=False, stop=True)
                        nc.vector.tensor_copy(ot[:, q * 512:(q + 1) * 512],
                                              ps[:])
                    nc.sync.dma_start(d_out[sb * 128:(sb + 1) * 128, :], ot[:])

    nc.compile()
    return nc


_prog = None


def _get_prog():
    global _prog
    if _prog is None:
        _prog = build_program()
    return _prog


def _host_prep(x, prior_mask, prior_indices, prior_index_mask, u_prev,
               Wq, bq, Wk, bk, Wv, bv, Wo, bo):
    f32 = np.float32
    x = np.asarray(x, f32)
    pm = np.asarray(prior_mask, bool)
    idx = np.asarray(prior_indices)
    pim = np.asarray(prior_index_mask, bool)
    u = np.asarray(u_prev, f32).reshape(B)
    Wq, Wk, Wv, Wo = (np.asarray(w, f32) for w in (Wq, Wk, Wv, Wo))
    bq, bk, bv, bo = (np.asarray(v, f32) for v in (bq, bk, bv, bo))

    scale = f32(1.0 / np.sqrt(DH))
    lam = (LAMBDA_MAX * np.exp(-ALPHA * u.astype(np.float64))).astype(f32)
    use_sparse = lam >= SPARSE_THRESHOLD

    # Sparse multiplicity bias (shared across batches): log(count) or -1e4.
    bts_sparse = None
    if use_sparse.any():
        cnt = np.zeros((S, S + 1), np.int32)
        np.add.at(cnt, (np.arange(S)[:, None],
                        np.where(pim, idx, S).astype(np.int64)), 1)
        cnt = cnt[:, :S]
        bsp = np.where(cnt > 0, np.log(np.maximum(cnt, 1)).astype(f32),
                       f32(-10000.0))
        bts_sparse = np.ascontiguousarray(bsp.T)

    bts = []
    for b in range(B):
        if use_sparse[b]:
            bts.append(bts_sparse)
        else:
            bd = np.where(pm, f32(0.0), f32(-lam[b]))
            bts.append(np.ascontiguousarray(bd.T))

    in_maps = []
    for c in range(N_CORES):
        b = c // 2
        hg = c % 2
        hsl = slice(hg * 512, (hg + 1) * 512)
        in_maps.append({
            "xt": np.ascontiguousarray(x[b].T),
            "wqt": np.ascontiguousarray((Wq[hsl] * scale).T),
            "wkt": np.ascontiguousarray(Wk[hsl].T),
            "wvt": np.ascontiguousarray(Wv[hsl].T),
            "wot": np.ascontiguousarray(Wo[:, hsl].T),
            "bt": bts[b],
            "bq": np.ascontiguousarray((bq[hsl] * scale).reshape(4, 128).T),
            "bk": np.ascontiguousarray(bk[hsl].reshape(4, 128).T),
            "bv": np.ascontiguousarray(bv[hsl].reshape(1, 512)),
            "bo": np.ascontiguousarray((bo * f32(0.5)).reshape(1, D)),
            "ident": np.eye(128, dtype=f32),
            "ones": np.ones((128, 512), dtype=f32),
        })
    return in_maps


def kernel(**inputs):
    in_maps = _host_prep(**inputs)
    nc = _get_prog()
    res = run_bass_kernel_spmd(nc, in_maps, core_ids=list(range(N_CORES)))
    out = np.empty((B, S, D), np.float32)
    for b in range(B):
        out[b] = res.results[2 * b]["out"] + res.results[2 * b + 1]["out"]
    return out


# revision 12
# speedup vs baseline: 1.3063x; 1.0637x over previous
"""Trainium2 Bass kernel for ClippingAttentionEngine.

Sharding: core c -> (batch b = c//2, head-group hg = c%2, 8 heads each).
Each core computes Q/K/V projections for its 8 heads, attention, and the
partial output projection over its head slice; host sums the two per-batch
partials (tensor-parallel over heads, per the sharding hint).

The per-batch sparse/dense branch is folded into a single dense-shaped
program: a host-built additive bias matrix B encodes either the dense
prior bias (0 / -lambda) or, for sparse batches, log(multiplicity) of each
key under prior_indices/prior_index_mask (-1e4 where never indexed), which
makes dense softmax(QK^T*scale + B) @ V exactly reproduce the gathered
sparse softmax (duplicates included).

Device pipeline per core (all matmuls float32r):
  A) xT/W streamed in; Q^T,K^T ([d',s] layout) and V (natural, with a
     ones-column per head for the softmax denominator) projected.
  B) per (head-pair, q-half) group: scores^T = B^T (via identity matmul)
     + K^T.T @ Q^T accumulated in PSUM, exp on ACT into P^T tiles; the
     attn@V' accumulation + normalization of the PREVIOUS group is
     software-pipelined against the current group's scores.
  C) partial out-projection from the normalized attn^T tiles.
"""

import sys

sys.path.insert(0, "/opt/trn_rl_repo")

import numpy as np

import concourse.bass as bass
import concourse.tile as tile
from concourse import bacc, mybir
from concourse.alu_op_type import AluOpType
from concourse.bass_utils import run_bass_kernel_spmd

B, S, D, H = 4, 1024, 1024, 16
DH = D // H          # 64
HPC = 8              # heads per core
N_CORES = 8
KT = S // 128        # 8 k tiles
DCH = D // 128       # 8 contraction chunks
LAMBDA_MAX, ALPHA, SPARSE_THRESHOLD = 10.0, 5.0, 1.0

F32 = mybir.dt.float32
F32R = mybir.dt.float32r
EXP = mybir.ActivationFunctionType.Exp
IDENT = mybir.ActivationFunctionType.Identity


def build_program():
    nc = bacc.Bacc("TRN2", target_bir_lowering=False, debug=False,
                   num_devices=N_CORES)

    d_xt = nc.dram_tensor("xt", [D, S], F32R, kind="ExternalInput").ap()
    d_wqt = nc.dram_tensor("wqt", [D, 512], F32R, kind="ExternalInput").ap()
    d_wkt = nc.dram_tensor("wkt", [D, 512], F32R, kind="ExternalInput").ap()
    d_wvt = nc.dram_tensor("wvt", [D, 512], F32R, kind="ExternalInput").ap()
    d_wot = nc.dram_tensor("wot", [512, D], F32R, kind="ExternalInput").ap()
    d_bt = nc.dram_tensor("bt", [S, S], F32R, kind="ExternalInput").ap()
    d_bq = nc.dram_tensor("bq", [128, 4], F32, kind="ExternalInput").ap()
    d_bk = nc.dram_tensor("bk", [128, 4], F32, kind="ExternalInput").ap()
    d_bv = nc.dram_tensor("bv", [1, 512], F32R, kind="ExternalInput").ap()
    d_bo = nc.dram_tensor("bo", [1, D], F32R, kind="ExternalInput").ap()
    d_id = nc.dram_tensor("ident", [128, 128], F32R, kind="ExternalInput").ap()
    d_ones = nc.dram_tensor("ones", [128, 512], F32R, kind="ExternalInput").ap()
    d_out = nc.dram_tensor("out", [S, D], F32, kind="ExternalOutput").ap()

    with tile.TileContext(nc) as tc:
        with (
            tc.tile_pool(name="const", bufs=1) as constp,
            tc.tile_pool(name="main", bufs=1) as mainp,
        ):
            ident = constp.tile([128, 128], F32R, tag="ident")
            nc.sync.dma_start(ident[:], d_id[:])
            ones = constp.tile([1, 512], F32R, tag="ones")
            nc.sync.dma_start(ones[:], d_ones[0:1, :])
            onecol = constp.tile([128, 8], F32R, tag="onecol")
            nc.sync.dma_start(onecol[:], d_ones[:, 0:8])
            bq_sb = constp.tile([128, 4], F32, tag="bq")
            nc.sync.dma_start(bq_sb[:], d_bq[:])
            bk_sb = constp.tile([128, 4], F32, tag="bk")
            nc.sync.dma_start(bk_sb[:], d_bk[:])
            bv_sb = constp.tile([1, 512], F32R, tag="bv")
            nc.sync.dma_start(bv_sb[:], d_bv[:])
            bo_sb = constp.tile([1, D], F32R, tag="bo")
            nc.sync.dma_start(bo_sb[:], d_bo[:])

            # Persistent arrays.
            qt_sb = [mainp.tile([128, S], F32R, tag=f"qt{m}", name=f"qt{m}")
                     for m in range(4)]
            kt_sb = [mainp.tile([128, S], F32R, tag=f"kt{m}", name=f"kt{m}")
                     for m in range(4)]
            vp_sb = [mainp.tile([128, HPC * (DH + 1)], F32R, tag=f"vp{sb}",
                                name=f"vp{sb}") for sb in range(8)]
            at_sb = [mainp.tile([128, S], F32R, tag=f"at{m}", name=f"at{m}")
                     for m in range(4)]
            wot_sb = [mainp.tile([128, D], F32R, tag=f"wot{mc}", name=f"wot{mc}")
                      for mc in range(4)]
            bt_sb = [mainp.tile([128, S], F32R, tag=f"bt{k}", name=f"bt{k}")
                     for k in range(KT)]
            # ---- Stage A: projections (scoped: xT + W slices + wide psum) ----
            with (
                tc.tile_pool(name="stageA", bufs=1) as pA,
                tc.tile_pool(name="ppp", bufs=2, space="PSUM") as ppp,
            ):
                xt_sb = [pA.tile([128, S], F32R, tag=f"xt{c}", name=f"xt{c}")
                         for c in range(DCH)]
                for c in range(DCH):
                    nc.sync.dma_start(xt_sb[c][:], d_xt[c * 128:(c + 1) * 128, :])
                w_sb = {}
                for nm, dap in (("q", d_wqt), ("k", d_wkt), ("v", d_wvt)):
                    w_sb[nm] = [pA.tile([128, 512], F32R, tag=f"w{nm}{c}",
                                        name=f"w{nm}{c}") for c in range(DCH)]
                    for c in range(DCH):
                        nc.sync.dma_start(w_sb[nm][c][:],
                                          dap[c * 128:(c + 1) * 128, :])

                # Q^T / K^T: psum[d'128, s1024] = sum_c W^T[c][:,d'].T @ xT[c]
                for nm, dst, bias in (("q", qt_sb, bq_sb), ("k", kt_sb, bk_sb)):
                    for m in range(4):
                        pp = ppp.tile([128, 1024], F32, tag="pp")
                        for st in range(2):
                            for c in range(DCH):
                                nc.tensor.matmul(
                                    pp[:, st * 512:(st + 1) * 512],
                                    w_sb[nm][c][:, m * 128:(m + 1) * 128],
                                    xt_sb[c][:, st * 512:(st + 1) * 512],
                                    start=(c == 0), stop=(c == DCH - 1))
                        nc.scalar.activation(dst[m][:], pp[:],
                                             IDENT, bias=bias[:, m:m + 1])

                # V natural: psum[s128, dh512] = sum_c xT[c][:,sblk].T @ WvT[c]
                for sb in range(8):
                    pp = ppp.tile([128, 1024], F32, tag="pp")
                    ps = pp[:, 0:512]
                    for c in range(DCH):
                        nc.tensor.matmul(
                            ps,
                            xt_sb[c][:, sb * 128:(sb + 1) * 128],
                            w_sb["v"][c][:],
                            start=(c == 0), stop=False)
                    nc.tensor.matmul(ps, ones[0:1, 0:128], bv_sb[:],
                                     start=False, stop=True)
                    vp3 = vp_sb[sb].rearrange("p (h d) -> p h d", d=DH + 1)
                    nc.vector.tensor_copy(
                        vp3[:, :, 0:DH],
                        ps.rearrange("p (h d) -> p h d", d=DH))
                    nc.vector.tensor_copy(
                        vp3[:, :, DH:DH + 1],
                        onecol[:].rearrange("p (h o) -> p h o", o=1))

            for k in range(KT):
                nc.sync.dma_start(bt_sb[k][:], d_bt[k * 128:(k + 1) * 128, :])
            for mc in range(4):
                nc.sync.dma_start(wot_sb[mc][:], d_wot[mc * 128:(mc + 1) * 128, :])

            # ---- Stage B: attention, software-pipelined by (pair, q-half) --
            with (
                tc.tile_pool(name="ptp", bufs=28) as ptp,
                tc.tile_pool(name="smallp", bufs=2) as smallp,
                tc.tile_pool(name="outp", bufs=2) as outp,
                tc.tile_pool(name="psS", bufs=4, space="PSUM") as psS,
                tc.tile_pool(name="psO", bufs=4, space="PSUM") as psO,
            ):
                def emit_scores(m, q):
                    pts = {}
                    for k in range(KT):
                        pss = []
                        for hh in range(2):
                            ps = psS.tile([128, 512], F32, tag="ps",
                                          name=f"ps{hh}")
                            nc.tensor.matmul(
                                ps[:], ident[:],
                                bt_sb[k][:, q * 512:(q + 1) * 512],
                                start=True, stop=False)
                            pss.append(ps)
                        for hh in range(2):
                            nc.tensor.matmul(
                                pss[hh][:],
                                kt_sb[m][hh * 64:(hh + 1) * 64,
                                         k * 128:(k + 1) * 128],
                                qt_sb[m][hh * 64:(hh + 1) * 64,
                                         q * 512:(q + 1) * 512],
                                start=False, stop=True,
                                tile_position=(hh * 64, 0))
                        for hh in range(2):
                            pt = ptp.tile([128, 512], F32R, tag="pt",
                                          name=f"pt{hh}")
                            nc.scalar.activation(pt[:], pss[hh][:], EXP)
                            pts[(hh, k)] = pt
                    return pts

                def emit_attnv(m, q, pts):
                    pos = []
                    for hh in range(2):
                        h = m * 2 + hh
                        po = psO.tile([DH + 1, 512], F32, tag="att",
                                      name=f"po{hh}")
                        for k in range(KT):
                            nc.tensor.matmul(
                                po[:],
                                vp_sb[k][:, h * (DH + 1):(h + 1) * (DH + 1)],
                                pts[(hh, k)][:],
                                start=(k == 0), stop=(k == KT - 1))
                        pos.append(po)
                    for hh in range(2):
                        zrow = smallp.tile([1, 512], F32, tag="zrow",
                                           name=f"zr{hh}")
                        nc.vector.tensor_copy(zrow[:], pos[hh][DH:DH + 1, :])
                        rec = smallp.tile([1, 512], F32, tag="rec",
                                          name=f"rc{hh}")
                        scr = smallp.tile([1, 512], F32, tag="scr",
                                          name=f"sc{hh}")
                        nc.vector.reciprocal_approx_accurate(rec[:], zrow[:],
                                                             scr[:])
                        bc = smallp.tile([64, 512], F32, tag="bc",
                                         name=f"bc{hh}")
                        nc.gpsimd.partition_broadcast(bc[:], rec[:])
                        nc.vector.tensor_tensor(
                            at_sb[m][hh * 64:(hh + 1) * 64,
                                     q * 512:(q + 1) * 512],
                            pos[hh][0:DH, :], bc[:], AluOpType.mult)

                groups = [(m, q) for m in range(4) for q in range(2)]
                prev = None
                for g in groups:
                    pts = emit_scores(*g)
                    if prev is not None:
                        emit_attnv(prev[0][0], prev[0][1], prev[1])
                    prev = (g, pts)
                emit_attnv(prev[0][0], prev[0][1], prev[1])

                # ---- Stage C: partial output projection ----
                for sb in range(8):
                    ot = outp.tile([128, D], F32, tag="ot")
                    for q in range(2):
                        ps = psS.tile([128, 512], F32, tag="ps")
                        for mc in range(4):
                            nc.tensor.matmul(
                                ps[:],
                                at_sb[mc][:, sb * 128:(sb + 1) * 128],
                                wot_sb[mc][:, q * 512:(q + 1) * 512],
                                start=(mc == 0), stop=False)
                        nc.tensor.matmul(ps[:], ones[0:1, 0:128],
                                         bo_sb[0:1, q * 512:(q + 1) * 512],
                                         start=False, stop=True)
                        nc.vector.tensor_copy(ot[:, q * 512:(q + 1) * 512],
                                              ps[:])
                    nc.sync.dma_start(d_out[sb * 128:(sb + 1) * 128, :], ot[:])

    nc.compile()
    return nc


_prog = None


def _get_prog():
    global _prog
    if _prog is None:
        _prog = build_program()
    return _prog


def _host_prep(x, prior_mask, prior_indices, prior_index_mask, u_prev,
               Wq, bq, Wk, bk, Wv, bv, Wo, bo):
    f32 = np.float32
    x = np.asarray(x, f32)
    pm = np.asarray(prior_mask, bool)
    idx = np.asarray(prior_indices)
    pim = np.asarray(prior_index_mask, bool)
    u = np.asarray(u_prev, f32).reshape(B)
    Wq, Wk, Wv, Wo = (np.asarray(w, f32) for w in (Wq, Wk, Wv, Wo))
    bq, bk, bv, bo = (np.asarray(v, f32) for v in (bq, bk, bv, bo))

    scale = f32(1.0 / np.sqrt(DH))
    lam = (LAMBDA_MAX * np.exp(-ALPHA * u.astype(np.float64))).astype(f32)
    use_sparse = lam >= SPARSE_THRESHOLD

    # Sparse multiplicity bias (shared across batches): log(count) or -1e4.
    bts_sparse = None
    if use_sparse.any():
        cnt = np.zeros((S, S + 1), np.int32)
        np.add.at(cnt, (np.arange(S)[:, None],
                        np.where(pim, idx, S).astype(np.int64)), 1)
        cnt = cnt[:, :S]
        bsp = np.where(cnt > 0, np.log(np.maximum(cnt, 1)).astype(f32),
                       f32(-10000.0))
        bts_sparse = np.ascontiguousarray(bsp.T)

    bts = []
    for b in range(B):
        if use_sparse[b]:
            bts.append(bts_sparse)
        else:
            bd = np.where(pm, f32(0.0), f32(-lam[b]))
            bts.append(np.ascontiguousarray(bd.T))

    in_maps = []
    for c in range(N_CORES):
        b = c // 2
        hg = c % 2
        hsl = slice(hg * 512, (hg + 1) * 512)
        in_maps.append({
            "xt": np.ascontiguousarray(x[b].T),
            "wqt": np.ascontiguousarray((Wq[hsl] * scale).T),
            "wkt": np.ascontiguousarray(Wk[hsl].T),
            "wvt": np.ascontiguousarray(Wv[hsl].T),
            "wot": np.ascontiguousarray(Wo[:, hsl].T),
            "bt": bts[b],
            "bq": np.ascontiguousarray((bq[hsl] * scale).reshape(4, 128).T),
            "bk": np.ascontiguousarray(bk[hsl].reshape(4, 128).T),
            "bv": np.ascontiguousarray(bv[hsl].reshape(1, 512)),
            "bo": np.ascontiguousarray((bo * f32(0.5)).reshape(1, D)),
            "ident": np.eye(128, dtype=f32),
            "ones": np.ones((128, 512), dtype=f32),
        })
    return in_maps


def kernel(**inputs):
    in_maps = _host_prep(**inputs)
    nc = _get_prog()
    res = run_bass_kernel_spmd(nc, in_maps, core_ids=list(range(N_CORES)))
    out = np.empty((B, S, D), np.float32)
    for b in range(B):
        out[b] = res.results[2 * b]["out"] + res.results[2 * b + 1]["out"]
    return out
